# revision 23
# baseline (speedup 1.0000x reference)
"""Trainium2 Bass kernel for DeformableSubspaceModulatedConv2d.

Contract: kernel(**inputs) takes FULL unsharded inputs (as produced by
setup_inputs) and returns the FULL output [16, 512, 64, 64] f32.

Strategy (data-parallel over batch, 2 samples per core on 8 cores),
Winograd F(2x2, 3x3):
  host prep: per-sample weight pipeline is tiny (rank-8 basis delta,
    style modulation, demodulation, G-transform: <0.1% of module FLOPs)
    and is folded into input prep as f32 BLAS, producing per-sample
    transformed weights U[b, i, 16uv, o] bf16. The x-direction (column)
    stage of the Winograd input transform is likewise folded into the
    input layout: f4[b, ch, ib, i, 18rows, 4cls, 32tc] bf16.
  device per core (2 samples): for each sample / chunk of 16 output
    rows: DVE row-stage input transform f4 -> V tiles (k-major so PE
    streams immediately); PE matmuls 16 (u,v)-classes x 4 ob x 4 ib at
    256 cols accumulating over i into PSUM (4 groups in flight = all 8
    banks); ACT evacuates PSUM -> Mst bf16; DVE inverse transform
    (A^T M A) -> out tiles; DMA out.
  pipelining: vt(next chunk) is emitted before matmuls(cur) so the
    in-order DVE stream runs ahead of the PE; matmul groups are u-outer
    so V tiles release early (ob-outer on the final chunk so the last
    inverse transforms overlap the remaining groups); U is loaded as
    per-(ib,u) subtiles (u=0 split per-ob) so the first groups start
    after ~1.7MB of DMA; chunk (0,0) ships as raw columns with the
    column stage on the idle DVE during the ramp; sample 1's U reloads
    in place, with WAR deps staggered across the last sample-0 chunk;
    a warm-up matmul burst holds the PE HAM clock-gate open during the
    initial DMA ramp.
  host: reassemble [y,x,tile] layout to [o,h,w], cast f32.
"""

import sys

sys.path.insert(0, "/opt/trn_rl_repo")

import numpy as np
import ml_dtypes
from contextlib import ExitStack

import concourse.bass as bass
import concourse.tile as tile
from concourse import bacc, bass_utils, mybir

F32 = mybir.dt.float32
BF16 = mybir.dt.bfloat16
AF = mybir.ActivationFunctionType

B, CIN, COUT, K, H, W = 16, 512, 512, 3, 64, 64
STYLE_DIM, BASIS, DIRS = 512, 8, 8
NCORES = 8
BLOC = B // NCORES  # 2 samples per core
NIB = CIN // 128  # 4 i blocks
NOB = COUT // 128  # 4 o blocks
KK = K * K  # 9
NCH = 4  # tile chunks per sample (8 tile-rows each)
NTR = 8  # tile-rows per chunk
NTC = 32  # tile-cols
SCALE = 1.0 / np.sqrt(CIN * K * K)
# v (and l) natural index -> class-ordered slot: v in {0,3} -> {0,1}, {1,2} -> {2,3}
VSLOT = {0: 0, 3: 1, 1: 2, 2: 3}

_NC_CACHE = {}
_RUN_KWARGS = {}
_LAST_RESULT = {}


def _build():
    nc = bacc.Bacc("TRN2", target_bir_lowering=False, debug=False)

    # ---- DRAM tensors ----
    f4_d = nc.dram_tensor("xf4", [BLOC, NCH, NIB, 128, 18, 4, NTC], BF16, kind="ExternalInput")
    xr_d = nc.dram_tensor("xr0", [NIB, 128, 18, 66], BF16, kind="ExternalInput")
    u_d = nc.dram_tensor("u_in", [BLOC, NIB, 128, 16, COUT], BF16, kind="ExternalInput")
    out_d = nc.dram_tensor("out", [BLOC, NOB, NCH, 128, 2, 2, NTR, NTC], BF16, kind="ExternalOutput")

    with tile.TileContext(nc) as tc, ExitStack() as top:
        u_pool = top.enter_context(tc.tile_pool(name="u", bufs=1))
        f4_pool = top.enter_context(tc.tile_pool(name="f4", bufs=6))
        xr_pool = top.enter_context(tc.tile_pool(name="xr", bufs=4))
        warm_pool = top.enter_context(tc.tile_pool(name="warm", bufs=1))
        v_pool = top.enter_context(tc.tile_pool(name="v", bufs=22))
        mst_pool = top.enter_context(tc.tile_pool(name="mst", bufs=5))
        sy_pool = top.enter_context(tc.tile_pool(name="sy", bufs=2))
        it_pool = top.enter_context(tc.tile_pool(name="it", bufs=2))
        outt_pool = top.enter_context(tc.tile_pool(name="outt", bufs=3))
        pc_psum = top.enter_context(tc.tile_pool(name="pc", bufs=4, space="PSUM"))

        # per-(ib, u) U subtiles so matmul groups start as soon as their
        # u-slice has landed (instead of waiting for the full 2.1MB tile)
        u_t = {
            (ib, u): u_pool.tile(
                [128, 4, COUT], BF16, tag=f"u{ib}_{u}", name=f"u{ib}_{u}"
            )
            for ib in range(NIB)
            for u in range(1, 4)
        }

        u0q_t = {
            (ib, q): u_pool.tile(
                [128, 4, 128], BF16, tag=f"u{ib}_0q{q}", name=f"u{ib}_0q{q}"
            )
            for ib in range(NIB)
            for q in range(4)
        }

        def load_u_set(s, u):
            if u == 0:
                for q in range(4):
                    for ib in range(NIB):
                        nc.sync.dma_start(
                            u0q_t[(ib, q)][:],
                            u_d.ap()[s, ib][:, 0:4, q * 128 : q * 128 + 128],
                        )
            else:
                for ib in range(NIB):
                    nc.sync.dma_start(
                        u_t[(ib, u)][:], u_d.ap()[s, ib][:, 4 * u : 4 * u + 4, :]
                    )

        def load_u_all(s):
            for u in range(4):
                load_u_set(s, u)

        def load_f4_chunk(s, ch):
            f4s = []
            for ib in range(NIB):
                f4 = f4_pool.tile([128, 18, 4, NTC], BF16, tag="f4")
                nc.sync.dma_start(f4[:], f4_d.ap()[s, ch, ib])
                f4s.append(f4[:].rearrange("p (rp two) l c -> p rp two l c", two=2))
            return f4s

        def load_raw_chunk00():
            """Chunk (0,0) ships as raw columns (half the bytes of f4) and the
            column stage runs on the otherwise-idle DVE during the ramp."""
            f4s = []
            for ib in range(NIB):
                xr = xr_pool.tile([128, 18, 66], BF16, tag="xr")
                nc.sync.dma_start(xr[:], xr_d.ap()[ib])
                xv = xr[:].rearrange("p r (two c) -> p r two c", two=2)
                f4 = f4_pool.tile([128, 18, 4, NTC], BF16, tag="f4")
                E0 = xv[:, :, 0, 0:32]
                E1 = xv[:, :, 0, 1:33]
                O0 = xv[:, :, 1, 0:32]
                O1 = xv[:, :, 1, 1:33]
                nc.vector.tensor_sub(f4[:, :, 0, :], E0, E1)
                nc.vector.tensor_add(f4[:, :, 1, :], O0, E1)
                nc.vector.tensor_sub(f4[:, :, 2, :], E1, O0)
                nc.vector.tensor_sub(f4[:, :, 3, :], O0, O1)
                f4s.append(f4[:].rearrange("p (rp two) l c -> p rp two l c", two=2))
            return f4s

        def vt_chunk(f4s):
            v_t = {}
            for k in range(4):
                for ib in range(NIB):
                    fr = f4s[ib]
                    r1 = fr[:, 0:8, 1, :, :]
                    r2 = fr[:, 1:9, 0, :, :]
                    vt = v_pool.tile([128, NTR, 4, NTC], BF16, tag="v")
                    if k == 0:
                        nc.vector.tensor_sub(vt[:], fr[:, 0:8, 0, :, :], r2)
                    elif k == 1:
                        nc.vector.tensor_add(vt[:], r1, r2)
                    elif k == 2:
                        nc.vector.tensor_sub(vt[:], r2, r1)
                    else:
                        nc.vector.tensor_sub(vt[:], r1, fr[:, 1:9, 1, :, :])
                    v_t[(ib, k)] = vt
            return v_t

        def matmul_chunk(v_t, ob_outer=False):
            """u-outer: V tiles of class u are released after u's 4 ob groups.
            ob_outer (last chunk): each ob's mst completes early so its
            inverse transform overlaps the remaining groups."""
            msts = [
                mst_pool.tile([128, 4, 4, 256], BF16, tag="mst", name=f"mst{ob}")
                for ob in range(NOB)
            ]
            order = (
                [(u, ob) for ob in range(NOB) for u in range(4)]
                if ob_outer
                else [(u, ob) for u in range(4) for ob in range(NOB)]
            )
            for u, ob in order:
                psq = pc_psum.tile([128, 4, 256], F32, tag="pc")
                for vpair in ((0, 1), (3, 2)):
                    for ib in range(NIB):
                        for v in vpair:
                            slot = VSLOT[v]
                            lhsT = (
                                u0q_t[(ib, ob)][:, slot, :]
                                if u == 0
                                else u_t[(ib, u)][:, slot, ob * 128 : ob * 128 + 128]
                            )
                            nc.tensor.matmul(
                                psq[:, slot, :],
                                lhsT,
                                v_t[(ib, u)][:, :, v, :],
                                start=(ib == 0),
                                stop=(ib == NIB - 1),
                            )
                nc.scalar.activation(msts[ob][:, u, :, :], psq[:], AF.Copy)
            return msts

        def inverse_chunk(s, ch, msts):
            for ob in range(NOB):
                mst = msts[ob]
                # y-stage over u classes
                sy = sy_pool.tile([128, 2, 4, 256], BF16, tag="sy")
                tmp1 = it_pool.tile([128, 4, 256], BF16, tag="it1")
                nc.vector.tensor_add(tmp1[:], mst[:, 0, :, :], mst[:, 1, :, :])
                nc.vector.tensor_add(sy[:, 0, :, :], tmp1[:], mst[:, 2, :, :])
                tmp2 = it_pool.tile([128, 4, 256], BF16, tag="it1", name="tmp2")
                nc.vector.tensor_sub(tmp2[:], mst[:, 1, :, :], mst[:, 2, :, :])
                nc.vector.tensor_sub(sy[:, 1, :, :], tmp2[:], mst[:, 3, :, :])
                # x-stage over v classes
                outt = outt_pool.tile([128, 2, 2, NTR, NTC], BF16, tag="outt")
                l0, l1, l2, l3 = VSLOT[0], VSLOT[1], VSLOT[2], VSLOT[3]
                ta = it_pool.tile([128, 2, 256], BF16, tag="ita")
                nc.vector.tensor_add(ta[:], sy[:, :, l1, :], sy[:, :, l2, :])
                ov0 = outt[:, :, 0, :, :].rearrange("p y r c -> p y (r c)")
                nc.vector.tensor_add(ov0, ta[:], sy[:, :, l0, :])
                tb = it_pool.tile([128, 2, 256], BF16, tag="ita", name="tb")
                nc.vector.tensor_sub(tb[:], sy[:, :, l1, :], sy[:, :, l2, :])
                ov1 = outt[:, :, 1, :, :].rearrange("p y r c -> p y (r c)")
                nc.vector.tensor_sub(ov1, tb[:], sy[:, :, l3, :])
                nc.sync.dma_start(out_d.ap()[s, ob, ch], outt[:])

        # software pipeline: emit vt(next) before matmuls(cur)/inverse(cur) so
        # the in-order DVE stream runs ahead of the PE across chunk boundaries
        f4s = load_raw_chunk00()
        # u0..u2 upfront; u3 after the next f4 chunk in queue order so the
        # early f4 chunks are not stuck behind the whole U stream
        load_u_set(0, 0)
        load_u_set(0, 1)
        load_u_set(0, 2)
        # HAM warm-up: keep the PE busy during the initial DMA ramp so the
        # clock gate is at 8/8 when the first real matmul groups issue
        warm = warm_pool.tile([128, 128], BF16, tag="warm")
        nc.vector.memset(warm[:], 0.0)
        # preload the ACT function table during the ramp so chunk 0's first
        # evacuation does not pay the one-time ACT_TABLE_LOAD (~1.3us)
        wact = warm_pool.tile([128, 1], BF16, tag="wact")
        nc.scalar.activation(wact[:], warm[:, 0:1], AF.Copy)
        for _ in range(120):
            psw = pc_psum.tile([128, 128], F32, tag="pc", name="warmps")
            nc.tensor.matmul(psw[:], warm[:], warm[:], start=True, stop=True)
        cur_v = vt_chunk(f4s)
        work = [(0, ch) for ch in range(NCH)] + [(1, ch) for ch in range(NCH)]
        for idx, (s, ch) in enumerate(work):
            if idx == 0:
                load_u_set(0, 3)
            nxt_v = None
            if idx + 1 < len(work):
                s2, ch2 = work[idx + 1]
                f4s = load_f4_chunk(s2, ch2)
                nxt_v = vt_chunk(f4s)
            msts = matmul_chunk(cur_v, ob_outer=(idx == len(work) - 1))
            if (s, ch) == (0, NCH - 1):
                # sample-1 U reload, u-major, emitted after the last sample-0
                # matmul reads; each subtile's WAR clears at its last conv-0
                # read (mid final chunk), hiding the swap
                load_u_all(1)
            inverse_chunk(s, ch, msts)
            cur_v = nxt_v

    nc.compile()
    return nc


def _get_nc():
    if "nc" not in _NC_CACHE:
        _NC_CACHE["nc"] = _build()
    return _NC_CACHE["nc"]


# Winograd F(2x2,3x3) weight transform matrix
_G = np.array(
    [[1.0, 0.0, 0.0], [0.5, 0.5, 0.5], [0.5, -0.5, 0.5], [0.0, 0.0, 1.0]],
    dtype=np.float32,
)


def kernel(**inputs):
    x = np.asarray(inputs["x"], dtype=np.float32)
    style = np.asarray(inputs["style"], dtype=np.float32)
    weight = np.asarray(inputs["weight"], dtype=np.float32)
    mod_w = np.asarray(inputs["mod_w"], dtype=np.float32)
    mod_b = np.asarray(inputs["mod_b"], dtype=np.float32)
    bv = np.asarray(inputs["basis_vectors"], dtype=np.float32)
    shifts_coords = np.asarray(inputs["shifts_coords"], dtype=np.float32)
    batch_shifts = np.asarray(inputs["batch_shifts"], dtype=np.float32)
    batch_directions = np.asarray(inputs["batch_directions"])

    # ---- per-sample weight pipeline (f32 host BLAS; <0.1% of module FLOPs) ----
    coefs = shifts_coords[batch_directions].astype(np.float64)  # [B, 8]
    bvf = bv.reshape(BASIS, -1)
    G8 = (bvf.astype(np.float64)) @ (bvf.astype(np.float64).T)
    nrm2 = np.einsum("bi,ij,bj->b", coefs, G8, coefs)
    nrm = np.sqrt(np.maximum(nrm2, 0.0))
    kfac = batch_shifts.astype(np.float64) / np.maximum(nrm, 1e-12)
    ck = (coefs * kfac[:, None]).astype(np.float32)  # [B, 8]

    s = style @ mod_w.T + mod_b  # [B, I]
    # W1[b] = weight + k_b * sum_j c_bj bv_j     [B, O, I, 9]
    W1 = (ck @ bvf).reshape(B, COUT, CIN, KK)
    W1 += weight[0].reshape(1, COUT, CIN, KK)
    # demod_b,o = rsqrt(sum_{i,kk} (scale * W1 * s_i)^2 + 1e-8)
    t9 = np.einsum("boik,boik->boi", W1, W1)
    dsum = (SCALE * SCALE) * np.einsum("boi,bi->bo", t9, s * s)
    demod = 1.0 / np.sqrt(dsum + 1e-8)
    # fold scale, style modulation and demod into the weights
    wfin = W1 * (SCALE * s)[:, None, :, None] * demod[:, :, None, None]
    del W1, t9
    # Winograd transform: U = G' Wf G'^T over the 3x3 kernel dims
    wfin = wfin.reshape(B * COUT * CIN, 3, 3)
    t = wfin.reshape(-1, 3) @ _G.T  # [.., 3(kr), 4(v)]
    t = np.ascontiguousarray(t.reshape(-1, 3, 4).transpose(0, 2, 1))  # [.., 4(v), 3(kr)]
    U = t.reshape(-1, 3) @ _G.T  # [.., 4(v), 4(u)]
    U = U.reshape(B, COUT, CIN, 4, 4)
    # reorder v-axis to slot order [v0, v3, v1, v2]; layout [b, ib, i, u*4+vs, o]
    U = U[:, :, :, [0, 3, 1, 2], :]  # [B, O, I, vs, u]
    U = np.ascontiguousarray(U.transpose(0, 2, 4, 3, 1))  # [B, I, u, vs, O]
    U_bf = U.reshape(B, NIB, 128, 16, COUT).astype(ml_dtypes.bfloat16)

    # ---- input layout + Winograd column stage (host) ----
    xp = np.zeros((B, CIN, H + 2, W + 2), dtype=np.float32)
    xp[:, :, 1 : H + 1, 1 : W + 1] = x
    E = xp[:, :, :, 0::2]  # [B, C, 66, 33]
    O = xp[:, :, :, 1::2]
    f4 = np.empty((B, CIN, H + 2, 4, NTC), dtype=np.float32)
    f4[:, :, :, 0, :] = E[:, :, :, 0:32] - E[:, :, :, 1:33]
    f4[:, :, :, 1, :] = O[:, :, :, 0:32] + E[:, :, :, 1:33]
    f4[:, :, :, 2, :] = E[:, :, :, 1:33] - O[:, :, :, 0:32]
    f4[:, :, :, 3, :] = O[:, :, :, 0:32] - O[:, :, :, 1:33]
    f4 = f4.astype(ml_dtypes.bfloat16)
    # chunk slabs of 18 rows (overlap 2)
    xs = np.stack([f4[:, :, 16 * ch : 16 * ch + 18] for ch in range(NCH)], axis=1)
    xs = np.ascontiguousarray(xs.reshape(B, NCH, NIB, 128, 18, 4, NTC))
    # raw deinterleaved slab for chunk (0,0) of each core's first sample:
    # [B, C, 18, 2, 33] -> [B, NIB, 128, 18, 66]
    xr = xp[:, :, 0:18].reshape(B, CIN, 18, 33, 2).transpose(0, 1, 2, 4, 3)
    xr = np.ascontiguousarray(xr).astype(ml_dtypes.bfloat16)
    xr = xr.reshape(B, NIB, 128, 18, 66)

    in_maps = []
    for c in range(NCORES):
        sl = slice(c * BLOC, (c + 1) * BLOC)
        in_maps.append(
            {
                "xf4": np.ascontiguousarray(xs[sl]),
                "xr0": np.ascontiguousarray(xr[c * BLOC]),
                "u_in": np.ascontiguousarray(U_bf[sl]),
            }
        )

    nc = _get_nc()
    res = bass_utils.run_bass_kernel_spmd(
        nc, in_maps, core_ids=list(range(NCORES)), **_RUN_KWARGS
    )
    _LAST_RESULT["res"] = res
    # reassemble: out_d [BLOC, NOB, NCH, 128, 2y, 2x, 8tr, 32tc]
    outs = []
    for c in range(NCORES):
        o = np.asarray(res.results[c]["out"]).astype(np.float32)
        # -> [b, ob, o128, ch, tr, y, tc, x] -> [b, 512, 64, 64]
        o = o.transpose(0, 1, 3, 2, 6, 4, 7, 5).reshape(BLOC, COUT, H, W)
        outs.append(o)
    return np.concatenate(outs, axis=0)


# revision 24
# speedup vs baseline: 1.1805x; 1.1805x over previous
"""Trainium2 Bass kernel for DeformableSubspaceModulatedConv2d.

Contract: kernel(**inputs) takes FULL unsharded inputs (as produced by
setup_inputs) and returns the FULL output [16, 512, 64, 64] f32.

Strategy (data-parallel over batch, 2 samples per core on 8 cores),
Winograd F(2x2, 3x3):
  host prep: per-sample weight pipeline is tiny (rank-8 basis delta,
    style modulation, demodulation, G-transform: <0.1% of module FLOPs)
    and is folded into input prep as f32 BLAS, producing per-sample
    transformed weights U[b, i, 16uv, o] bf16. The x-direction (column)
    stage of the Winograd input transform is likewise folded into the
    input layout: f4[b, ch, ib, i, 18rows, 4cls, 32tc] bf16.
  device per core (2 samples): for each sample / chunk of 16 output
    rows: DVE row-stage input transform f4 -> V tiles (k-major so PE
    streams immediately); PE matmuls 16 (u,v)-classes x 4 ob x 4 ib at
    256 cols accumulating over i into PSUM (4 groups in flight = all 8
    banks); ACT evacuates PSUM -> Mst bf16; DVE inverse transform
    (A^T M A) -> out tiles; DMA out.
  pipelining: vt(next chunk) is emitted before matmuls(cur) so the
    in-order DVE stream runs ahead of the PE; matmul groups are u-outer
    so V tiles release early (ob-outer on the final chunk so the last
    inverse transforms overlap the remaining groups); U is loaded as
    per-(ib,u) subtiles (u=0 split per-ob) so the first groups start
    after ~1.7MB of DMA; chunk (0,0) ships as raw columns with the
    column stage on the idle DVE during the ramp; sample 1's U reloads
    in place, with WAR deps staggered across the last sample-0 chunk;
    a warm-up matmul burst holds the PE HAM clock-gate open during the
    initial DMA ramp.
  host: reassemble [y,x,tile] layout to [o,h,w], cast f32.
"""

import sys

sys.path.insert(0, "/opt/trn_rl_repo")

import numpy as np
import ml_dtypes
from contextlib import ExitStack

import concourse.bass as bass
import concourse.tile as tile
from concourse import bacc, bass_utils, mybir

F32 = mybir.dt.float32
BF16 = mybir.dt.bfloat16
AF = mybir.ActivationFunctionType

B, CIN, COUT, K, H, W = 16, 512, 512, 3, 64, 64
STYLE_DIM, BASIS, DIRS = 512, 8, 8
NCORES = 8
BLOC = B // NCORES  # 2 samples per core
NIB = CIN // 128  # 4 i blocks
NOB = COUT // 128  # 4 o blocks
KK = K * K  # 9
NCH = 4  # tile chunks per sample (8 tile-rows each)
NTR = 8  # tile-rows per chunk
NTC = 32  # tile-cols
SCALE = 1.0 / np.sqrt(CIN * K * K)
# v (and l) natural index -> class-ordered slot: v in {0,3} -> {0,1}, {1,2} -> {2,3}
VSLOT = {0: 0, 3: 1, 1: 2, 2: 3}

_NC_CACHE = {}
_RUN_KWARGS = {}
_LAST_RESULT = {}


def _build():
    nc = bacc.Bacc("TRN2", target_bir_lowering=False, debug=False)

    # ---- DRAM tensors ----
    f4_d = nc.dram_tensor("xf4", [BLOC, NCH, NIB, 128, 18, 4, NTC], BF16, kind="ExternalInput")
    xr_d = nc.dram_tensor("xr0", [NIB, 128, 18, 66], BF16, kind="ExternalInput")
    u_d = nc.dram_tensor("u_in", [BLOC, NIB, 128, 16, COUT], BF16, kind="ExternalInput")
    out_d = nc.dram_tensor("out", [BLOC, NOB, NCH, 128, 2, 2, NTR, NTC], BF16, kind="ExternalOutput")

    with tile.TileContext(nc) as tc, ExitStack() as top:
        u_pool = top.enter_context(tc.tile_pool(name="u", bufs=1))
        f4_pool = top.enter_context(tc.tile_pool(name="f4", bufs=6))
        xr_pool = top.enter_context(tc.tile_pool(name="xr", bufs=4))
        warm_pool = top.enter_context(tc.tile_pool(name="warm", bufs=1))
        v_pool = top.enter_context(tc.tile_pool(name="v", bufs=22))
        mst_pool = top.enter_context(tc.tile_pool(name="mst", bufs=5))
        sy_pool = top.enter_context(tc.tile_pool(name="sy", bufs=2))
        it_pool = top.enter_context(tc.tile_pool(name="it", bufs=2))
        outt_pool = top.enter_context(tc.tile_pool(name="outt", bufs=3))
        pc_psum = top.enter_context(tc.tile_pool(name="pc", bufs=4, space="PSUM"))

        # per-(ib, u) U subtiles so matmul groups start as soon as their
        # u-slice has landed (instead of waiting for the full 2.1MB tile)
        u_t = {
            (ib, u): u_pool.tile(
                [128, 4, COUT], BF16, tag=f"u{ib}_{u}", name=f"u{ib}_{u}"
            )
            for ib in range(NIB)
            for u in range(1, 4)
        }

        u0q_t = {
            (ib, q): u_pool.tile(
                [128, 4, 128], BF16, tag=f"u{ib}_0q{q}", name=f"u{ib}_0q{q}"
            )
            for ib in range(NIB)
            for q in range(4)
        }

        def load_u_set(s, u):
            if u == 0:
                for q in range(4):
                    for ib in range(NIB):
                        nc.sync.dma_start(
                            u0q_t[(ib, q)][:],
                            u_d.ap()[s, ib][:, 0:4, q * 128 : q * 128 + 128],
                        )
            else:
                for ib in range(NIB):
                    nc.sync.dma_start(
                        u_t[(ib, u)][:], u_d.ap()[s, ib][:, 4 * u : 4 * u + 4, :]
                    )

        def load_u_all(s):
            for u in range(4):
                load_u_set(s, u)

        def load_f4_chunk(s, ch):
            f4s = []
            for ib in range(NIB):
                f4 = f4_pool.tile([128, 18, 4, NTC], BF16, tag="f4")
                nc.sync.dma_start(f4[:], f4_d.ap()[s, ch, ib])
                f4s.append(f4[:].rearrange("p (rp two) l c -> p rp two l c", two=2))
            return f4s

        def load_raw_chunk00():
            """Chunk (0,0) ships as raw columns (half the bytes of f4) and the
            column stage runs on the otherwise-idle DVE during the ramp."""
            f4s = []
            for ib in range(NIB):
                xr = xr_pool.tile([128, 18, 66], BF16, tag="xr")
                nc.sync.dma_start(xr[:], xr_d.ap()[ib])
                xv = xr[:].rearrange("p r (two c) -> p r two c", two=2)
                f4 = f4_pool.tile([128, 18, 4, NTC], BF16, tag="f4")
                E0 = xv[:, :, 0, 0:32]
                E1 = xv[:, :, 0, 1:33]
                O0 = xv[:, :, 1, 0:32]
                O1 = xv[:, :, 1, 1:33]
                nc.vector.tensor_sub(f4[:, :, 0, :], E0, E1)
                nc.vector.tensor_add(f4[:, :, 1, :], O0, E1)
                nc.vector.tensor_sub(f4[:, :, 2, :], E1, O0)
                nc.vector.tensor_sub(f4[:, :, 3, :], O0, O1)
                f4s.append(f4[:].rearrange("p (rp two) l c -> p rp two l c", two=2))
            return f4s

        def vt_chunk(f4s):
            v_t = {}
            for k in range(4):
                for ib in range(NIB):
                    fr = f4s[ib]
                    r1 = fr[:, 0:8, 1, :, :]
                    r2 = fr[:, 1:9, 0, :, :]
                    vt = v_pool.tile([128, NTR, 4, NTC], BF16, tag="v")
                    if k == 0:
                        nc.vector.tensor_sub(vt[:], fr[:, 0:8, 0, :, :], r2)
                    elif k == 1:
                        nc.vector.tensor_add(vt[:], r1, r2)
                    elif k == 2:
                        nc.vector.tensor_sub(vt[:], r2, r1)
                    else:
                        nc.vector.tensor_sub(vt[:], r1, fr[:, 1:9, 1, :, :])
                    v_t[(ib, k)] = vt
            return v_t

        def matmul_chunk(v_t, ob_outer=False):
            """u-outer: V tiles of class u are released after u's 4 ob groups.
            ob_outer (last chunk): each ob's mst completes early so its
            inverse transform overlaps the remaining groups."""
            msts = [
                mst_pool.tile([128, 4, 4, 256], BF16, tag="mst", name=f"mst{ob}")
                for ob in range(NOB)
            ]
            order = (
                [(u, ob) for ob in range(NOB) for u in range(4)]
                if ob_outer
                else [(u, ob) for u in range(4) for ob in range(NOB)]
            )
            for u, ob in order:
                psq = pc_psum.tile([128, 4, 256], F32, tag="pc")
                for vpair in ((0, 1), (3, 2)):
                    for ib in range(NIB):
                        for v in vpair:
                            slot = VSLOT[v]
                            lhsT = (
                                u0q_t[(ib, ob)][:, slot, :]
                                if u == 0
                                else u_t[(ib, u)][:, slot, ob * 128 : ob * 128 + 128]
                            )
                            nc.tensor.matmul(
                                psq[:, slot, :],
                                lhsT,
                                v_t[(ib, u)][:, :, v, :],
                                start=(ib == 0),
                                stop=(ib == NIB - 1),
                            )
                nc.scalar.activation(msts[ob][:, u, :, :], psq[:], AF.Copy)
            return msts

        def inverse_chunk(s, ch, msts):
            for ob in range(NOB):
                mst = msts[ob]
                # y-stage over u classes
                sy = sy_pool.tile([128, 2, 4, 256], BF16, tag="sy")
                tmp1 = it_pool.tile([128, 4, 256], BF16, tag="it1")
                nc.vector.tensor_add(tmp1[:], mst[:, 0, :, :], mst[:, 1, :, :])
                nc.vector.tensor_add(sy[:, 0, :, :], tmp1[:], mst[:, 2, :, :])
                tmp2 = it_pool.tile([128, 4, 256], BF16, tag="it1", name="tmp2")
                nc.vector.tensor_sub(tmp2[:], mst[:, 1, :, :], mst[:, 2, :, :])
                nc.vector.tensor_sub(sy[:, 1, :, :], tmp2[:], mst[:, 3, :, :])
                # x-stage over v classes
                outt = outt_pool.tile([128, 2, 2, NTR, NTC], BF16, tag="outt")
                l0, l1, l2, l3 = VSLOT[0], VSLOT[1], VSLOT[2], VSLOT[3]
                ta = it_pool.tile([128, 2, 256], BF16, tag="ita")
                nc.vector.tensor_add(ta[:], sy[:, :, l1, :], sy[:, :, l2, :])
                ov0 = outt[:, :, 0, :, :].rearrange("p y r c -> p y (r c)")
                nc.vector.tensor_add(ov0, ta[:], sy[:, :, l0, :])
                tb = it_pool.tile([128, 2, 256], BF16, tag="ita", name="tb")
                nc.vector.tensor_sub(tb[:], sy[:, :, l1, :], sy[:, :, l2, :])
                ov1 = outt[:, :, 1, :, :].rearrange("p y r c -> p y (r c)")
                nc.vector.tensor_sub(ov1, tb[:], sy[:, :, l3, :])
                nc.sync.dma_start(out_d.ap()[s, ob, ch], outt[:])

        # software pipeline: emit vt(next) before matmuls(cur)/inverse(cur) so
        # the in-order DVE stream runs ahead of the PE across chunk boundaries
        f4s = load_raw_chunk00()
        # u0..u2 upfront; u3 after the next f4 chunk in queue order so the
        # early f4 chunks are not stuck behind the whole U stream
        load_u_set(0, 0)
        load_u_set(0, 1)
        load_u_set(0, 2)
        # HAM warm-up: keep the PE busy during the initial DMA ramp so the
        # clock gate is at 8/8 when the first real matmul groups issue
        warm = warm_pool.tile([128, 128], BF16, tag="warm")
        nc.vector.memset(warm[:], 0.0)
        for _ in range(120):
            psw = pc_psum.tile([128, 128], F32, tag="pc", name="warmps")
            nc.tensor.matmul(psw[:], warm[:], warm[:], start=True, stop=True)
        cur_v = vt_chunk(f4s)
        work = [(0, ch) for ch in range(NCH)] + [(1, ch) for ch in range(NCH)]
        for idx, (s, ch) in enumerate(work):
            if idx == 0:
                load_u_set(0, 3)
            nxt_v = None
            if idx + 1 < len(work):
                s2, ch2 = work[idx + 1]
                f4s = load_f4_chunk(s2, ch2)
                nxt_v = vt_chunk(f4s)
            msts = matmul_chunk(cur_v, ob_outer=(idx == len(work) - 1))
            if (s, ch) == (0, NCH - 1):
                # sample-1 U reload, u-major, emitted after the last sample-0
                # matmul reads; each subtile's WAR clears at its last conv-0
                # read (mid final chunk), hiding the swap
                load_u_all(1)
            inverse_chunk(s, ch, msts)
            cur_v = nxt_v

    nc.compile()
    return nc


def _get_nc():
    if "nc" not in _NC_CACHE:
        _NC_CACHE["nc"] = _build()
    return _NC_CACHE["nc"]


# Winograd F(2x2,3x3) weight transform matrix
_G = np.array(
    [[1.0, 0.0, 0.0], [0.5, 0.5, 0.5], [0.5, -0.5, 0.5], [0.0, 0.0, 1.0]],
    dtype=np.float32,
)


def kernel(**inputs):
    x = np.asarray(inputs["x"], dtype=np.float32)
    style = np.asarray(inputs["style"], dtype=np.float32)
    weight = np.asarray(inputs["weight"], dtype=np.float32)
    mod_w = np.asarray(inputs["mod_w"], dtype=np.float32)
    mod_b = np.asarray(inputs["mod_b"], dtype=np.float32)
    bv = np.asarray(inputs["basis_vectors"], dtype=np.float32)
    shifts_coords = np.asarray(inputs["shifts_coords"], dtype=np.float32)
    batch_shifts = np.asarray(inputs["batch_shifts"], dtype=np.float32)
    batch_directions = np.asarray(inputs["batch_directions"])

    # ---- per-sample weight pipeline (f32 host BLAS; <0.1% of module FLOPs) ----
    coefs = shifts_coords[batch_directions].astype(np.float64)  # [B, 8]
    bvf = bv.reshape(BASIS, -1)
    G8 = (bvf.astype(np.float64)) @ (bvf.astype(np.float64).T)
    nrm2 = np.einsum("bi,ij,bj->b", coefs, G8, coefs)
    nrm = np.sqrt(np.maximum(nrm2, 0.0))
    kfac = batch_shifts.astype(np.float64) / np.maximum(nrm, 1e-12)
    ck = (coefs * kfac[:, None]).astype(np.float32)  # [B, 8]

    s = style @ mod_w.T + mod_b  # [B, I]
    # W1[b] = weight + k_b * sum_j c_bj bv_j     [B, O, I, 9]
    W1 = (ck @ bvf).reshape(B, COUT, CIN, KK)
    W1 += weight[0].reshape(1, COUT, CIN, KK)
    # demod_b,o = rsqrt(sum_{i,kk} (scale * W1 * s_i)^2 + 1e-8)
    t9 = np.einsum("boik,boik->boi", W1, W1)
    dsum = (SCALE * SCALE) * np.einsum("boi,bi->bo", t9, s * s)
    demod = 1.0 / np.sqrt(dsum + 1e-8)
    # fold scale, style modulation and demod into the weights
    wfin = W1 * (SCALE * s)[:, None, :, None] * demod[:, :, None, None]
    del W1, t9
    # Winograd transform: U = G' Wf G'^T over the 3x3 kernel dims
    wfin = wfin.reshape(B * COUT * CIN, 3, 3)
    t = wfin.reshape(-1, 3) @ _G.T  # [.., 3(kr), 4(v)]
    t = np.ascontiguousarray(t.reshape(-1, 3, 4).transpose(0, 2, 1))  # [.., 4(v), 3(kr)]
    U = t.reshape(-1, 3) @ _G.T  # [.., 4(v), 4(u)]
    U = U.reshape(B, COUT, CIN, 4, 4)
    # reorder v-axis to slot order [v0, v3, v1, v2]; layout [b, ib, i, u*4+vs, o]
    U = U[:, :, :, [0, 3, 1, 2], :]  # [B, O, I, vs, u]
    U = np.ascontiguousarray(U.transpose(0, 2, 4, 3, 1))  # [B, I, u, vs, O]
    U_bf = U.reshape(B, NIB, 128, 16, COUT).astype(ml_dtypes.bfloat16)

    # ---- input layout + Winograd column stage (host) ----
    xp = np.zeros((B, CIN, H + 2, W + 2), dtype=np.float32)
    xp[:, :, 1 : H + 1, 1 : W + 1] = x
    E = xp[:, :, :, 0::2]  # [B, C, 66, 33]
    O = xp[:, :, :, 1::2]
    f4 = np.empty((B, CIN, H + 2, 4, NTC), dtype=np.float32)
    f4[:, :, :, 0, :] = E[:, :, :, 0:32] - E[:, :, :, 1:33]
    f4[:, :, :, 1, :] = O[:, :, :, 0:32] + E[:, :, :, 1:33]
    f4[:, :, :, 2, :] = E[:, :, :, 1:33] - O[:, :, :, 0:32]
    f4[:, :, :, 3, :] = O[:, :, :, 0:32] - O[:, :, :, 1:33]
    f4 = f4.astype(ml_dtypes.bfloat16)
    # chunk slabs of 18 rows (overlap 2)
    xs = np.stack([f4[:, :, 16 * ch : 16 * ch + 18] for ch in range(NCH)], axis=1)
    xs = np.ascontiguousarray(xs.reshape(B, NCH, NIB, 128, 18, 4, NTC))
    # raw deinterleaved slab for chunk (0,0) of each core's first sample:
    # [B, C, 18, 2, 33] -> [B, NIB, 128, 18, 66]
    xr = xp[:, :, 0:18].reshape(B, CIN, 18, 33, 2).transpose(0, 1, 2, 4, 3)
    xr = np.ascontiguousarray(xr).astype(ml_dtypes.bfloat16)
    xr = xr.reshape(B, NIB, 128, 18, 66)

    in_maps = []
    for c in range(NCORES):
        sl = slice(c * BLOC, (c + 1) * BLOC)
        in_maps.append(
            {
                "xf4": np.ascontiguousarray(xs[sl]),
                "xr0": np.ascontiguousarray(xr[c * BLOC]),
                "u_in": np.ascontiguousarray(U_bf[sl]),
            }
        )

    nc = _get_nc()
    res = bass_utils.run_bass_kernel_spmd(
        nc, in_maps, core_ids=list(range(NCORES)), **_RUN_KWARGS
    )
    _LAST_RESULT["res"] = res
    # reassemble: out_d [BLOC, NOB, NCH, 128, 2y, 2x, 8tr, 32tc]
    outs = []
    for c in range(NCORES):
        o = np.asarray(res.results[c]["out"]).astype(np.float32)
        # -> [b, ob, o128, ch, tr, y, tc, x] -> [b, 512, 64, 64]
        o = o.transpose(0, 1, 3, 2, 6, 4, 7, 5).reshape(BLOC, COUT, H, W)
        outs.append(o)
    return np.concatenate(outs, axis=0)


# revision 25
# speedup vs baseline: 1.1874x; 1.0058x over previous
"""Trainium2 Bass kernel for DeformableSubspaceModulatedConv2d.

Contract: kernel(**inputs) takes FULL unsharded inputs (as produced by
setup_inputs) and returns the FULL output [16, 512, 64, 64] f32.

Strategy (data-parallel over batch, 2 samples per core on 8 cores),
Winograd F(2x2, 3x3):
  host prep: per-sample weight pipeline is tiny (rank-8 basis delta,
    style modulation, demodulation, G-transform: <0.1% of module FLOPs)
    and is folded into input prep as f32 BLAS, producing per-sample
    transformed weights U[b, i, 16uv, o] bf16. The x-direction (column)
    stage of the Winograd input transform is likewise folded into the
    input layout: f4[b, ch, ib, i, 18rows, 4cls, 32tc] bf16.
  device per core (2 samples): for each sample / chunk of 16 output
    rows: DVE row-stage input transform f4 -> V tiles (k-major so PE
    streams immediately); PE matmuls 16 (u,v)-classes x 4 ob x 4 ib at
    256 cols accumulating over i into PSUM (4 groups in flight = all 8
    banks); ACT evacuates PSUM -> Mst bf16; DVE inverse transform
    (A^T M A) -> out tiles; DMA out.
  pipelining: vt(next chunk) is emitted before matmuls(cur) so the
    in-order DVE stream runs ahead of the PE; matmul groups are u-outer
    so V tiles release early (ob-outer on the final chunk so the last
    inverse transforms overlap the remaining groups); U is loaded as
    per-(ib,u) subtiles (u=0 split per-ob) so the first groups start
    after ~1.7MB of DMA; chunk (0,0) ships as raw columns with the
    column stage on the idle DVE during the ramp; sample 1's U reloads
    in place, with WAR deps staggered across the last sample-0 chunk;
    a warm-up matmul burst holds the PE HAM clock-gate open during the
    initial DMA ramp.
  host: reassemble [y,x,tile] layout to [o,h,w], cast f32.
"""

import sys

sys.path.insert(0, "/opt/trn_rl_repo")

import numpy as np
import ml_dtypes
from contextlib import ExitStack

import concourse.bass as bass
import concourse.tile as tile
from concourse import bacc, bass_utils, mybir

F32 = mybir.dt.float32
BF16 = mybir.dt.bfloat16
AF = mybir.ActivationFunctionType

B, CIN, COUT, K, H, W = 16, 512, 512, 3, 64, 64
STYLE_DIM, BASIS, DIRS = 512, 8, 8
NCORES = 8
BLOC = B // NCORES  # 2 samples per core
NIB = CIN // 128  # 4 i blocks
NOB = COUT // 128  # 4 o blocks
KK = K * K  # 9
NCH = 4  # tile chunks per sample (8 tile-rows each)
NTR = 8  # tile-rows per chunk
NTC = 32  # tile-cols
SCALE = 1.0 / np.sqrt(CIN * K * K)
# v (and l) natural index -> class-ordered slot: v in {0,3} -> {0,1}, {1,2} -> {2,3}
VSLOT = {0: 0, 3: 1, 1: 2, 2: 3}

_NC_CACHE = {}
_RUN_KWARGS = {}
_LAST_RESULT = {}


def _build():
    nc = bacc.Bacc("TRN2", target_bir_lowering=False, debug=False)

    # ---- DRAM tensors ----
    f4_d = nc.dram_tensor("xf4", [BLOC, NCH, NIB, 128, 18, 4, NTC], BF16, kind="ExternalInput")
    xr_d = nc.dram_tensor("xr0", [NIB, 128, 18, 66], BF16, kind="ExternalInput")
    u_d = nc.dram_tensor("u_in", [BLOC, NIB, 128, 16, COUT], BF16, kind="ExternalInput")
    out_d = nc.dram_tensor("out", [BLOC, NOB, NCH, 128, 2, 2, NTR, NTC], BF16, kind="ExternalOutput")

    with tile.TileContext(nc) as tc, ExitStack() as top:
        u_pool = top.enter_context(tc.tile_pool(name="u", bufs=1))
        f4_pool = top.enter_context(tc.tile_pool(name="f4", bufs=6))
        xr_pool = top.enter_context(tc.tile_pool(name="xr", bufs=4))
        warm_pool = top.enter_context(tc.tile_pool(name="warm", bufs=1))
        v_pool = top.enter_context(tc.tile_pool(name="v", bufs=22))
        mst_pool = top.enter_context(tc.tile_pool(name="mst", bufs=5))
        sy_pool = top.enter_context(tc.tile_pool(name="sy", bufs=2))
        it_pool = top.enter_context(tc.tile_pool(name="it", bufs=2))
        outt_pool = top.enter_context(tc.tile_pool(name="outt", bufs=3))
        pc_psum = top.enter_context(tc.tile_pool(name="pc", bufs=4, space="PSUM"))

        # per-(ib, u) U subtiles so matmul groups start as soon as their
        # u-slice has landed (instead of waiting for the full 2.1MB tile)
        u_t = {
            (ib, u): u_pool.tile(
                [128, 4, COUT], BF16, tag=f"u{ib}_{u}", name=f"u{ib}_{u}"
            )
            for ib in range(NIB)
            for u in range(2, 4)
        }

        u0q_t = {
            (ib, q): u_pool.tile(
                [128, 4, 128], BF16, tag=f"u{ib}_0q{q}", name=f"u{ib}_0q{q}"
            )
            for ib in range(NIB)
            for q in range(4)
        }

        u1h_t = {
            (ib, h): u_pool.tile(
                [128, 4, 256], BF16, tag=f"u{ib}_1h{h}", name=f"u{ib}_1h{h}"
            )
            for ib in range(NIB)
            for h in range(2)
        }

        def load_u_set(s, u):
            if u == 0:
                for q in range(4):
                    for ib in range(NIB):
                        nc.sync.dma_start(
                            u0q_t[(ib, q)][:],
                            u_d.ap()[s, ib][:, 0:4, q * 128 : q * 128 + 128],
                        )
            elif u == 1:
                for h in range(2):
                    for ib in range(NIB):
                        nc.sync.dma_start(
                            u1h_t[(ib, h)][:],
                            u_d.ap()[s, ib][:, 4:8, h * 256 : h * 256 + 256],
                        )
            else:
                for ib in range(NIB):
                    nc.sync.dma_start(
                        u_t[(ib, u)][:], u_d.ap()[s, ib][:, 4 * u : 4 * u + 4, :]
                    )

        def load_u_all(s):
            for u in range(4):
                load_u_set(s, u)

        def load_f4_chunk(s, ch):
            f4s = []
            for ib in range(NIB):
                f4 = f4_pool.tile([128, 18, 4, NTC], BF16, tag="f4")
                nc.sync.dma_start(f4[:], f4_d.ap()[s, ch, ib])
                f4s.append(f4[:].rearrange("p (rp two) l c -> p rp two l c", two=2))
            return f4s

        def load_raw_chunk00():
            """Chunk (0,0) ships as raw columns (half the bytes of f4) and the
            column stage runs on the otherwise-idle DVE during the ramp."""
            f4s = []
            for ib in range(NIB):
                xr = xr_pool.tile([128, 18, 66], BF16, tag="xr")
                nc.sync.dma_start(xr[:], xr_d.ap()[ib])
                xv = xr[:].rearrange("p r (two c) -> p r two c", two=2)
                f4 = f4_pool.tile([128, 18, 4, NTC], BF16, tag="f4")
                E0 = xv[:, :, 0, 0:32]
                E1 = xv[:, :, 0, 1:33]
                O0 = xv[:, :, 1, 0:32]
                O1 = xv[:, :, 1, 1:33]
                nc.vector.tensor_sub(f4[:, :, 0, :], E0, E1)
                nc.vector.tensor_add(f4[:, :, 1, :], O0, E1)
                nc.vector.tensor_sub(f4[:, :, 2, :], E1, O0)
                nc.vector.tensor_sub(f4[:, :, 3, :], O0, O1)
                f4s.append(f4[:].rearrange("p (rp two) l c -> p rp two l c", two=2))
            return f4s

        def vt_chunk(f4s):
            v_t = {}
            for k in range(4):
                for ib in range(NIB):
                    fr = f4s[ib]
                    r1 = fr[:, 0:8, 1, :, :]
                    r2 = fr[:, 1:9, 0, :, :]
                    vt = v_pool.tile([128, NTR, 4, NTC], BF16, tag="v")
                    if k == 0:
                        nc.vector.tensor_sub(vt[:], fr[:, 0:8, 0, :, :], r2)
                    elif k == 1:
                        nc.vector.tensor_add(vt[:], r1, r2)
                    elif k == 2:
                        nc.vector.tensor_sub(vt[:], r2, r1)
                    else:
                        nc.vector.tensor_sub(vt[:], r1, fr[:, 1:9, 1, :, :])
                    v_t[(ib, k)] = vt
            return v_t

        def matmul_chunk(v_t, ob_outer=False):
            """u-outer: V tiles of class u are released after u's 4 ob groups.
            ob_outer (last chunk): each ob's mst completes early so its
            inverse transform overlaps the remaining groups."""
            msts = [
                mst_pool.tile([128, 4, 4, 256], BF16, tag="mst", name=f"mst{ob}")
                for ob in range(NOB)
            ]
            order = (
                [(u, ob) for ob in range(NOB) for u in range(4)]
                if ob_outer
                else [(u, ob) for u in range(4) for ob in range(NOB)]
            )
            for u, ob in order:
                psq = pc_psum.tile([128, 4, 256], F32, tag="pc")
                for vpair in ((0, 1), (3, 2)):
                    for ib in range(NIB):
                        for v in vpair:
                            slot = VSLOT[v]
                            if u == 0:
                                lhsT = u0q_t[(ib, ob)][:, slot, :]
                            elif u == 1:
                                lhsT = u1h_t[(ib, ob // 2)][
                                    :, slot, (ob % 2) * 128 : (ob % 2) * 128 + 128
                                ]
                            else:
                                lhsT = u_t[(ib, u)][:, slot, ob * 128 : ob * 128 + 128]
                            nc.tensor.matmul(
                                psq[:, slot, :],
                                lhsT,
                                v_t[(ib, u)][:, :, v, :],
                                start=(ib == 0),
                                stop=(ib == NIB - 1),
                            )
                nc.scalar.activation(msts[ob][:, u, :, :], psq[:], AF.Copy)
            return msts

        def inverse_chunk(s, ch, msts):
            for ob in range(NOB):
                mst = msts[ob]
                # y-stage over u classes
                sy = sy_pool.tile([128, 2, 4, 256], BF16, tag="sy")
                tmp1 = it_pool.tile([128, 4, 256], BF16, tag="it1")
                nc.vector.tensor_add(tmp1[:], mst[:, 0, :, :], mst[:, 1, :, :])
                nc.vector.tensor_add(sy[:, 0, :, :], tmp1[:], mst[:, 2, :, :])
                tmp2 = it_pool.tile([128, 4, 256], BF16, tag="it1", name="tmp2")
                nc.vector.tensor_sub(tmp2[:], mst[:, 1, :, :], mst[:, 2, :, :])
                nc.vector.tensor_sub(sy[:, 1, :, :], tmp2[:], mst[:, 3, :, :])
                # x-stage over v classes
                outt = outt_pool.tile([128, 2, 2, NTR, NTC], BF16, tag="outt")
                l0, l1, l2, l3 = VSLOT[0], VSLOT[1], VSLOT[2], VSLOT[3]
                ta = it_pool.tile([128, 2, 256], BF16, tag="ita")
                nc.vector.tensor_add(ta[:], sy[:, :, l1, :], sy[:, :, l2, :])
                ov0 = outt[:, :, 0, :, :].rearrange("p y r c -> p y (r c)")
                nc.vector.tensor_add(ov0, ta[:], sy[:, :, l0, :])
                tb = it_pool.tile([128, 2, 256], BF16, tag="ita", name="tb")
                nc.vector.tensor_sub(tb[:], sy[:, :, l1, :], sy[:, :, l2, :])
                ov1 = outt[:, :, 1, :, :].rearrange("p y r c -> p y (r c)")
                nc.vector.tensor_sub(ov1, tb[:], sy[:, :, l3, :])
                nc.sync.dma_start(out_d.ap()[s, ob, ch], outt[:])

        # software pipeline: emit vt(next) before matmuls(cur)/inverse(cur) so
        # the in-order DVE stream runs ahead of the PE across chunk boundaries
        f4s = load_raw_chunk00()
        # u0..u2 upfront; u3 after the next f4 chunk in queue order so the
        # early f4 chunks are not stuck behind the whole U stream
        load_u_set(0, 0)
        load_u_set(0, 1)
        load_u_set(0, 2)
        # HAM warm-up: keep the PE busy during the initial DMA ramp so the
        # clock gate is at 8/8 when the first real matmul groups issue
        warm = warm_pool.tile([128, 128], BF16, tag="warm")
        nc.vector.memset(warm[:], 0.0)
        for _ in range(120):
            psw = pc_psum.tile([128, 128], F32, tag="pc", name="warmps")
            nc.tensor.matmul(psw[:], warm[:], warm[:], start=True, stop=True)
        cur_v = vt_chunk(f4s)
        work = [(0, ch) for ch in range(NCH)] + [(1, ch) for ch in range(NCH)]
        for idx, (s, ch) in enumerate(work):
            if idx == 0:
                load_u_set(0, 3)
            nxt_v = None
            if idx + 1 < len(work):
                s2, ch2 = work[idx + 1]
                f4s = load_f4_chunk(s2, ch2)
                nxt_v = vt_chunk(f4s)
            msts = matmul_chunk(cur_v, ob_outer=(idx == len(work) - 1))
            if (s, ch) == (0, NCH - 1):
                # sample-1 U reload, u-major, emitted after the last sample-0
                # matmul reads; each subtile's WAR clears at its last conv-0
                # read (mid final chunk), hiding the swap
                load_u_all(1)
            inverse_chunk(s, ch, msts)
            cur_v = nxt_v

    nc.compile()
    return nc


def _get_nc():
    if "nc" not in _NC_CACHE:
        _NC_CACHE["nc"] = _build()
    return _NC_CACHE["nc"]


# Winograd F(2x2,3x3) weight transform matrix
_G = np.array(
    [[1.0, 0.0, 0.0], [0.5, 0.5, 0.5], [0.5, -0.5, 0.5], [0.0, 0.0, 1.0]],
    dtype=np.float32,
)


def kernel(**inputs):
    x = np.asarray(inputs["x"], dtype=np.float32)
    style = np.asarray(inputs["style"], dtype=np.float32)
    weight = np.asarray(inputs["weight"], dtype=np.float32)
    mod_w = np.asarray(inputs["mod_w"], dtype=np.float32)
    mod_b = np.asarray(inputs["mod_b"], dtype=np.float32)
    bv = np.asarray(inputs["basis_vectors"], dtype=np.float32)
    shifts_coords = np.asarray(inputs["shifts_coords"], dtype=np.float32)
    batch_shifts = np.asarray(inputs["batch_shifts"], dtype=np.float32)
    batch_directions = np.asarray(inputs["batch_directions"])

    # ---- per-sample weight pipeline (f32 host BLAS; <0.1% of module FLOPs) ----
    coefs = shifts_coords[batch_directions].astype(np.float64)  # [B, 8]
    bvf = bv.reshape(BASIS, -1)
    G8 = (bvf.astype(np.float64)) @ (bvf.astype(np.float64).T)
    nrm2 = np.einsum("bi,ij,bj->b", coefs, G8, coefs)
    nrm = np.sqrt(np.maximum(nrm2, 0.0))
    kfac = batch_shifts.astype(np.float64) / np.maximum(nrm, 1e-12)
    ck = (coefs * kfac[:, None]).astype(np.float32)  # [B, 8]

    s = style @ mod_w.T + mod_b  # [B, I]
    # W1[b] = weight + k_b * sum_j c_bj bv_j     [B, O, I, 9]
    W1 = (ck @ bvf).reshape(B, COUT, CIN, KK)
    W1 += weight[0].reshape(1, COUT, CIN, KK)
    # demod_b,o = rsqrt(sum_{i,kk} (scale * W1 * s_i)^2 + 1e-8)
    t9 = np.einsum("boik,boik->boi", W1, W1)
    dsum = (SCALE * SCALE) * np.einsum("boi,bi->bo", t9, s * s)
    demod = 1.0 / np.sqrt(dsum + 1e-8)
    # fold scale, style modulation and demod into the weights
    wfin = W1 * (SCALE * s)[:, None, :, None] * demod[:, :, None, None]
    del W1, t9
    # Winograd transform: U = G' Wf G'^T over the 3x3 kernel dims
    wfin = wfin.reshape(B * COUT * CIN, 3, 3)
    t = wfin.reshape(-1, 3) @ _G.T  # [.., 3(kr), 4(v)]
    t = np.ascontiguousarray(t.reshape(-1, 3, 4).transpose(0, 2, 1))  # [.., 4(v), 3(kr)]
    U = t.reshape(-1, 3) @ _G.T  # [.., 4(v), 4(u)]
    U = U.reshape(B, COUT, CIN, 4, 4)
    # reorder v-axis to slot order [v0, v3, v1, v2]; layout [b, ib, i, u*4+vs, o]
    U = U[:, :, :, [0, 3, 1, 2], :]  # [B, O, I, vs, u]
    U = np.ascontiguousarray(U.transpose(0, 2, 4, 3, 1))  # [B, I, u, vs, O]
    U_bf = U.reshape(B, NIB, 128, 16, COUT).astype(ml_dtypes.bfloat16)

    # ---- input layout + Winograd column stage (host) ----
    xp = np.zeros((B, CIN, H + 2, W + 2), dtype=np.float32)
    xp[:, :, 1 : H + 1, 1 : W + 1] = x
    E = xp[:, :, :, 0::2]  # [B, C, 66, 33]
    O = xp[:, :, :, 1::2]
    f4 = np.empty((B, CIN, H + 2, 4, NTC), dtype=np.float32)
    f4[:, :, :, 0, :] = E[:, :, :, 0:32] - E[:, :, :, 1:33]
    f4[:, :, :, 1, :] = O[:, :, :, 0:32] + E[:, :, :, 1:33]
    f4[:, :, :, 2, :] = E[:, :, :, 1:33] - O[:, :, :, 0:32]
    f4[:, :, :, 3, :] = O[:, :, :, 0:32] - O[:, :, :, 1:33]
    f4 = f4.astype(ml_dtypes.bfloat16)
    # chunk slabs of 18 rows (overlap 2)
    xs = np.stack([f4[:, :, 16 * ch : 16 * ch + 18] for ch in range(NCH)], axis=1)
    xs = np.ascontiguousarray(xs.reshape(B, NCH, NIB, 128, 18, 4, NTC))
    # raw deinterleaved slab for chunk (0,0) of each core's first sample:
    # [B, C, 18, 2, 33] -> [B, NIB, 128, 18, 66]
    xr = xp[:, :, 0:18].reshape(B, CIN, 18, 33, 2).transpose(0, 1, 2, 4, 3)
    xr = np.ascontiguousarray(xr).astype(ml_dtypes.bfloat16)
    xr = xr.reshape(B, NIB, 128, 18, 66)

    in_maps = []
    for c in range(NCORES):
        sl = slice(c * BLOC, (c + 1) * BLOC)
        in_maps.append(
            {
                "xf4": np.ascontiguousarray(xs[sl]),
                "xr0": np.ascontiguousarray(xr[c * BLOC]),
                "u_in": np.ascontiguousarray(U_bf[sl]),
            }
        )

    nc = _get_nc()
    res = bass_utils.run_bass_kernel_spmd(
        nc, in_maps, core_ids=list(range(NCORES)), **_RUN_KWARGS
    )
    _LAST_RESULT["res"] = res
    # reassemble: out_d [BLOC, NOB, NCH, 128, 2y, 2x, 8tr, 32tc]
    outs = []
    for c in range(NCORES):
        o = np.asarray(res.results[c]["out"]).astype(np.float32)
        # -> [b, ob, o128, ch, tr, y, tc, x] -> [b, 512, 64, 64]
        o = o.transpose(0, 1, 3, 2, 6, 4, 7, 5).reshape(BLOC, COUT, H, W)
        outs.append(o)
    return np.concatenate(outs, axis=0)
